# revision 3
# baseline (speedup 1.0000x reference)
"""Distributed Trainium2 kernel for the gated-adapter attention module.

Head-parallel tensor parallelism over 8 NeuronCores (4 heads each).
v2 redesign vs the first working kernel:
  * all f32->bf16 casts ride SWDGE cast-DMAs (gpsimd) instead of DVE,
  * x / weights are staged bf16 in DRAM and transposed via the DMA xbar
    so the PE runs a pure back-to-back matmul stream (HAM stays warm),
  * V stays resident in SBUF (no DRAM round trip),
  * wo is cooperatively staged: each core casts+PE-transposes 1/8 of wo
    and an AllGather materializes the full wo^T bf16 copy per core, so
    phase 3 does plain contiguous reads,
  * the causal mask is generated on-chip (single shared diagonal block),
  * attention output is written straight into the AllToAll input layout.
Compute bf16, f32 PSUM accumulation.
"""

import sys

sys.path.insert(0, "/opt/trn_rl_repo")

import numpy as np

import concourse.bass as bass
import concourse.mybir as mybir
import concourse.tile as tile
from concourse import bacc, bass_utils
from concourse.bass import ds, ts
from concourse.masks import make_identity

N_CORES = 8
B, S, D = 2, 2048, 4096
H = 32
HD = 128                      # head dim
H_LOC = H // N_CORES          # 4 heads per core
CH = H_LOC * HD               # 512 local channels
TOK = B * S                   # 4096 tokens
NK = D // 128                 # 32 contraction tiles
AL = 10                       # adapter length
TPC = TOK // N_CORES          # 512 tokens per core after AllToAll
NQC = S // 512                # 4 query chunks per sequence
NPAN = TOK // 256             # 16 token panels for QKV
SCALE = 1.0 / float(np.sqrt(HD))
BF = mybir.dt.bfloat16
F32 = mybir.dt.float32
EXP = mybir.ActivationFunctionType.Exp
TANH = mybir.ActivationFunctionType.Tanh
MULT = mybir.AluOpType.mult
ADD = mybir.AluOpType.add


def build():
    nc = bacc.Bacc("TRN2", target_bir_lowering=False, debug=False,
                   num_devices=N_CORES)
    x = nc.dram_tensor("x", [TOK, D], F32, kind="ExternalInput")
    wq = nc.dram_tensor("wq", [CH, D], F32, kind="ExternalInput")
    wk = nc.dram_tensor("wk", [CH, D], F32, kind="ExternalInput")
    wv = nc.dram_tensor("wv", [CH, D], F32, kind="ExternalInput")
    wo_col = nc.dram_tensor("wo_col", [D, CH], F32, kind="ExternalInput")
    gate = nc.dram_tensor("gate", [1, H_LOC], F32, kind="ExternalInput")
    adapter = nc.dram_tensor("adapter", [AL, D], F32, kind="ExternalInput")
    fcos = nc.dram_tensor("fcos", [S, HD // 2], F32, kind="ExternalInput")
    fsin = nc.dram_tensor("fsin", [S, HD // 2], F32, kind="ExternalInput")
    out = nc.dram_tensor("out", [TPC, D], F32, kind="ExternalOutput")

    with tile.TileContext(nc) as tc:
        with tc.tile_pool(name="dram", bufs=1, space="DRAM") as dram, \
             tc.tile_pool(name="persist", bufs=1) as persist:
            xb_d = dram.tile([TOK, D], BF, tag="xb_d")
            wb_ds = [dram.tile([CH, D], BF, tag=f"wb{i}", name=f"wb{i}")
                     for i in range(3)]
            qn_ds = [dram.tile([S, CH], BF, tag=f"qn{b}", name=f"qn{b}")
                     for b in range(B)]
            kn_ds = [dram.tile([S, CH], BF, tag=f"kn{b}", name=f"kn{b}")
                     for b in range(B)]
            woT_my = dram.tile([CH, D], BF, tag="woT_my")
            woT_full = dram.tile([D, D], BF, tag="woT_full",
                                 addr_space="Shared")
            a2a_in = dram.tile([N_CORES, CH, TPC], BF, tag="a2a_in")
            a2a_out = dram.tile([N_CORES, CH, TPC], BF, tag="a2a_out")

            # --- staging cast-DMAs first so SWDGE queues start draining ---
            for p_i, wt in ((0, wq), (1, wk), (2, wv)):
                nc.gpsimd.dma_start(wb_ds[p_i][:, :], wt.ap())
            for pa in range(8):
                nc.gpsimd.dma_start(xb_d[ds(pa * 512, 512), :],
                                    x.ap()[ds(pa * 512, 512), :])

            ident = persist.tile([128, 128], BF, tag="ident")
            make_identity(nc, ident[:])
            ones = persist.tile([128, 128], BF, tag="ones")
            nc.vector.memset(ones[:], 1.0)
            g_sb = persist.tile([128, H_LOC], F32, tag="g_sb")
            g_in = persist.tile([128, H_LOC], F32, tag="g_in")
            nc.scalar.dma_start(g_in[:], gate.ap().partition_broadcast(128))
            nc.scalar.activation(g_sb[:], g_in[:], TANH)
            # single causal diagonal block, k on partitions, q on free dim:
            # keep 0 where q >= k else -1e30
            maskT = persist.tile([128, 128], BF, tag="maskT")
            nc.gpsimd.memset(maskT[:], 0.0)
            nc.gpsimd.affine_select(
                out=maskT[:], in_=maskT[:],
                compare_op=mybir.AluOpType.is_ge, fill=-1e30,
                base=0, pattern=[[1, 128]], channel_multiplier=-1)
            cs_all = persist.tile([128, S // 128, HD // 2], F32, tag="cs_all")
            nc.scalar.dma_start(
                cs_all[:], fcos.ap().rearrange("(pb p) f -> p pb f", p=128))
            sn_all = persist.tile([128, S // 128, HD // 2], F32, tag="sn_all")
            nc.scalar.dma_start(
                sn_all[:], fsin.ap().rearrange("(pb p) f -> p pb f", p=128))
            a_kT = persist.tile([128, H_LOC, AL], BF, tag="a_kT")
            a_v = persist.tile([AL, H_LOC, HD], BF, tag="a_v")
            v_all = persist.tile([128, TOK // 128, CH], BF, tag="v_all")

            # ============ phase 1: wo shard transpose + QKV ============
            with tc.tile_pool(name="wph", bufs=1) as wph, \
                 tc.tile_pool(name="pst", bufs=2, space="PSUM") as pst, \
                 tc.tile_pool(name="psb", bufs=2, space="PSUM") as psb:
                aT = persist.tile([128, NK, AL], BF, tag="aT")
                # wo shard: read f32 column slice, cast, PE-transpose into
                # woT_my, then AllGather the bf16 wo^T across cores.
                with tc.tile_pool(name="wos", bufs=2) as wos, \
                     tc.tile_pool(name="woa", bufs=1) as woa:
                    wo_asm = [woa.tile([128, D], BF, tag=f"woasm{cs}",
                                       name=f"woasm{cs}") for cs in range(4)]
                    for dt in range(NK):
                        wof = wos.tile([128, CH], F32, tag="wof")
                        nc.scalar.dma_start(wof[:],
                                            wo_col.ap()[ts(dt, 128), :])
                        wob = wos.tile([128, CH], BF, tag="wob")
                        nc.vector.tensor_copy(wob[:], wof[:])
                        for cs in range(4):
                            wps = pst.tile([128, 128], BF, tag="tps")
                            nc.tensor.transpose(wps[:], wob[:, ts(cs, 128)],
                                                ident[:])
                            nc.vector.tensor_copy(wo_asm[cs][:, ts(dt, 128)],
                                                  wps[:])
                    for cs in range(4):
                        nc.scalar.dma_start(woT_my[ds(cs * 128, 128), :],
                                            wo_asm[cs][:])
                nc.gpsimd.collective_compute(
                    "AllGather", mybir.AluOpType.bypass,
                    replica_groups=[list(range(N_CORES))],
                    ins=[woT_my.opt()], outs=[woT_full.opt()])

                # adapter^T [128 dim, AL] tiles (PE transpose, bf16)
                with tc.tile_pool(name="stg", bufs=2) as stg:
                    ab = stg.tile([AL, D], BF, tag="ab", bufs=1)
                    for hf in range(4):
                        af = stg.tile([AL, D // 4], F32, tag="af")
                        nc.scalar.dma_start(af[:],
                                            adapter.ap()[:, ts(hf, D // 4)])
                        nc.vector.tensor_copy(ab[:, ts(hf, D // 4)], af[:])
                    for dt in range(NK):
                        aps = pst.tile([128, 128], BF, tag="tps")
                        nc.tensor.transpose(aps[:, :AL], ab[:, ts(dt, 128)],
                                            ident[:AL, :AL])
                        nc.vector.tensor_copy(aT[:, dt, :], aps[:, :AL])

                # transposed weights [128, NK, CH] via DMA xbar
                wTs = []
                for p_i in range(3):
                    wT = wph.tile([128, NK, CH], BF, tag=f"wT{p_i}",
                                  name=f"wT{p_i}")
                    wTs.append(wT)
                    for dt in range(NK):
                        nc.sync.dma_start_transpose(
                            wT[:, dt, :], wb_ds[p_i][:, ts(dt, 128)])
                # a_k^T [ch, AL] per head, a_v [AL, ch]
                for cs in range(H_LOC):
                    pk = psb.tile([128, CH], F32, tag="ppq")
                    for dt in range(NK):
                        nc.tensor.matmul(pk[:, :AL],
                                         lhsT=wTs[1][:, dt, ts(cs, 128)],
                                         rhs=aT[:, dt, :], start=(dt == 0),
                                         stop=(dt == NK - 1))
                    nc.vector.tensor_copy(a_kT[:, cs, :], pk[:, :AL])
                pv = psb.tile([128, CH], F32, tag="ppq")
                for dt in range(NK):
                    nc.tensor.matmul(pv[:AL, :], lhsT=aT[:, dt, :],
                                     rhs=wTs[2][:, dt, :], start=(dt == 0),
                                     stop=(dt == NK - 1))
                for cs in range(H_LOC):
                    nc.vector.tensor_copy(a_v[:, cs, :], pv[:AL, ts(cs, 128)])

                # main QKV: panels of 256 tokens, xT via DMA xbar
                with tc.tile_pool(name="run", bufs=2) as st:
                    for pa in range(NPAN):
                        b_i = pa // (NPAN // B)
                        xT = st.tile([128, NK, 256], BF, tag="xT")
                        for dt in range(NK):
                            nc.sync.dma_start_transpose(
                                xT[:, dt, :],
                                xb_d[ds(pa * 256, 256), ts(dt, 128)])
                        for sp_i in range(2):
                            tstr = pa * 2 + sp_i
                            srow = (tstr % (S // 128)) * 128
                            pps = [psb.tile([128, CH], F32, tag=f"pp{pn}",
                                            name=f"pp{pn}") for pn in "qkv"]
                            for dt in range(NK):
                                for p_i in range(3):
                                    nc.tensor.matmul(
                                        pps[p_i][:],
                                        lhsT=xT[:, dt, ts(sp_i, 128)],
                                        rhs=wTs[p_i][:, dt, :],
                                        start=(dt == 0), stop=(dt == NK - 1))
                            # v: cast into resident SBUF tile
                            nc.vector.tensor_copy(v_all[:, tstr, :], pps[2][:])
                            # q, k: RoPE then store natural
                            csb = cs_all[:, srow // 128, :]
                            ssb = sn_all[:, srow // 128, :]
                            for p_i, dstl in ((0, qn_ds), (1, kn_ds)):
                                rp = st.tile([128, CH], BF, tag=f"rp{p_i}",
                                             name=f"rp{p_i}")
                                for h in range(H_LOC):
                                    pv2 = pps[p_i][:, ts(h, HD)].rearrange(
                                        "p (i two) -> p two i", two=2)
                                    rv = rp[:, ts(h, HD)].rearrange(
                                        "p (i two) -> p two i", two=2)
                                    a0, b0 = pv2[:, 0, :], pv2[:, 1, :]
                                    t1 = st.tile([128, HD // 2], F32, tag="t1")
                                    t2 = st.tile([128, HD // 2], F32, tag="t2")
                                    nc.vector.tensor_mul(t1[:], a0, csb)
                                    nc.vector.tensor_mul(t2[:], b0, ssb)
                                    nc.vector.tensor_sub(rv[:, 0, :],
                                                         t1[:], t2[:])
                                    nc.vector.tensor_mul(t1[:], a0, ssb)
                                    nc.vector.tensor_mul(t2[:], b0, csb)
                                    nc.vector.tensor_add(rv[:, 1, :],
                                                         t1[:], t2[:])
                                nc.scalar.dma_start(
                                    dstl[b_i][ds(srow, 128), :], rp[:])

            # ================= phase 2: attention =================
            with tc.tile_pool(name="at", bufs=2) as at, \
                 tc.tile_pool(name="att", bufs=3) as att, \
                 tc.tile_pool(name="ps_st", bufs=3, space="PSUM") as ps_st, \
                 tc.tile_pool(name="ps_ac", bufs=1, space="PSUM") as ps_ac:
                def _bh_loads(b_i, h):
                    qTb = at.tile([128, S], BF, tag="qTb", name="qTb")
                    nc.sync.dma_start_transpose(
                        qTb[:], qn_ds[b_i][:, ts(h, HD)])
                    kTb = at.tile([128, S], BF, tag="kTb", name="kTb")
                    nc.sync.dma_start_transpose(
                        kTb[:], kn_ds[b_i][:, ts(h, HD)])
                    return qTb, kTb

                cur = _bh_loads(0, 0)
                for bh in range(B * H_LOC):
                    b_i, h = divmod(bh, H_LOC)
                    nxt = (_bh_loads(*divmod(bh + 1, H_LOC))
                           if bh + 1 < B * H_LOC else None)
                    qTb, kTb = cur
                    for qc in range(NQC):
                        nkt = (qc + 1) * 4
                        stb = att.tile([128, S // 128, 512], BF, tag="stb",
                                       bufs=2)
                        for kt in range(nkt):
                            sps = ps_st.tile([128, 512], F32, tag="sps")
                            nc.tensor.matmul(sps[:],
                                             lhsT=kTb[:, ts(kt, 128)],
                                             rhs=qTb[:, ts(qc, 512)],
                                             start=True, stop=True)
                            if kt // 4 == qc:
                                off = (kt % 4) * 128
                                if off > 0:
                                    nc.vector.memset(
                                        stb[:, kt, ds(0, off)], 0.0)
                                sd = att.tile([128, 128], F32, tag="sd")
                                nc.vector.scalar_tensor_tensor(
                                    sd[:], sps[:, ds(off, 128)], SCALE,
                                    maskT[:], op0=MULT, op1=ADD)
                                nc.scalar.activation(
                                    stb[:, kt, ds(off, 128)], sd[:], EXP)
                                if off + 128 < 512:
                                    nc.scalar.activation(
                                        stb[:, kt,
                                            ds(off + 128, 384 - off)],
                                        sps[:, ds(off + 128, 384 - off)],
                                        EXP, scale=SCALE)
                            else:
                                nc.scalar.activation(stb[:, kt, :], sps[:],
                                                     EXP, scale=SCALE)
                        # adapter scores [AL, 512]
                        spa = ps_st.tile([128, 512], F32, tag="sps")
                        nc.tensor.matmul(spa[:AL, :], lhsT=a_kT[:, h, :],
                                         rhs=qTb[:, ts(qc, 512)],
                                         start=True, stop=True)
                        pab = att.tile([AL, 512], BF, tag="pab")
                        nc.scalar.activation(pab[:], spa[:AL, :], EXP,
                                             scale=SCALE)
                        # column sums via ones-matmul
                        s_ps = ps_ac.tile([1, 512], F32, tag="s_ps")
                        sa_ps = ps_ac.tile([1, 512], F32, tag="sa_ps")
                        for kt in range(nkt):
                            nc.tensor.matmul(s_ps[:], lhsT=ones[:, 0:1],
                                             rhs=stb[:, kt, :],
                                             start=(kt == 0),
                                             stop=(kt == nkt - 1))
                        nc.tensor.matmul(sa_ps[:], lhsT=ones[:AL, 0:1],
                                         rhs=pab[:], start=True, stop=True)
                        # PV accumulation: oT [128 d, 512 q]
                        o_ps = ps_ac.tile([128, 512], F32, tag="o_ps",
                                          bufs=2)
                        for kt in range(nkt):
                            nc.tensor.matmul(
                                o_ps[:],
                                lhsT=v_all[:, b_i * (S // 128) + kt,
                                           ts(h, HD)],
                                rhs=stb[:, kt, :],
                                start=(kt == 0), stop=(kt == nkt - 1))
                        oa_ps = ps_ac.tile([128, 512], F32, tag="oa_ps")
                        nc.tensor.matmul(oa_ps[:], lhsT=a_v[:, h, :],
                                         rhs=pab[:], start=True, stop=True)
                        # combine: o = o_main/s_main + tanh(g)*oa/s_adapt
                        sb2 = att.tile([1, 512], BF, tag="sb2")
                        nc.vector.tensor_copy(sb2[:], s_ps[:])
                        sb2a = att.tile([1, 512], BF, tag="sb2a")
                        nc.vector.tensor_copy(sb2a[:], sa_ps[:])
                        bc_ps = ps_st.tile([128, 512], F32, tag="sps")
                        nc.tensor.matmul(bc_ps[:], lhsT=ones[0:1, :],
                                         rhs=sb2[:], start=True, stop=True)
                        bca_ps = ps_st.tile([128, 512], F32, tag="sps")
                        nc.tensor.matmul(bca_ps[:], lhsT=ones[0:1, :],
                                         rhs=sb2a[:], start=True, stop=True)
                        rb = att.tile([128, 512], F32, tag="rb")
                        nc.vector.reciprocal_approx_fast(rb[:], bc_ps[:])
                        rba = att.tile([128, 512], F32, tag="rba")
                        nc.vector.reciprocal_approx_fast(rba[:], bca_ps[:])
                        t3 = att.tile([128, 512], F32, tag="t3")
                        nc.vector.tensor_mul(t3[:], o_ps[:], rb[:])
                        t4 = att.tile([128, 512], F32, tag="t4")
                        nc.vector.scalar_tensor_tensor(
                            t4[:], rba[:], g_sb[:, ds(h, 1)], oa_ps[:],
                            op0=MULT, op1=MULT)
                        ob = att.tile([128, 512], BF, tag="ob")
                        nc.vector.tensor_add(ob[:], t3[:], t4[:])
                        nc.scalar.dma_start(
                            a2a_in[b_i * NQC + qc][ds(h * HD, HD), :],
                            ob[:])
                    cur = nxt

            # ================= phase 3: AllToAll + wo =================
            nc.gpsimd.collective_compute(
                "AllToAll", mybir.AluOpType.bypass,
                replica_groups=[list(range(N_CORES))],
                ins=[a2a_in.opt()], outs=[a2a_out.opt()])
            with tc.tile_pool(name="wo_sb", bufs=3) as wsb, \
                 tc.tile_pool(name="wo_ps", bufs=1, space="PSUM") as wps, \
                 tc.tile_pool(name="of", bufs=1) as ofp:
                oTf = ofp.tile([128, NK, TPC], BF, tag="oTf")
                for sc in range(N_CORES):
                    nc.scalar.dma_start(
                        oTf[:, ds(sc * H_LOC, H_LOC), :],
                        a2a_out[sc].rearrange("(g p) t -> p g t", p=128))
                # 4 passes over d (1024 cols each); 8 psum banks = 4 tt x 2 d2
                for dp in range(4):
                    yps = [wps.tile([128, 512], F32, tag=f"yp{i}",
                                    name=f"yp{i}") for i in range(8)]
                    for et in range(NK):
                        wot = wsb.tile([128, 1024], BF, tag="wot")
                        nc.scalar.dma_start(
                            wot[:], woT_full[ts(et, 128), ts(dp, 1024)])
                        for tt in range(TPC // 128):
                            for d2 in range(2):
                                nc.tensor.matmul(
                                    yps[tt * 2 + d2][:],
                                    lhsT=oTf[:, et, ts(tt, 128)],
                                    rhs=wot[:, ts(d2, 512)],
                                    start=(et == 0), stop=(et == NK - 1))
                    for tt in range(TPC // 128):
                        for d2 in range(2):
                            yb = wsb.tile([128, 512], F32, tag="yb")
                            nc.vector.tensor_copy(yb[:], yps[tt * 2 + d2][:])
                            nc.scalar.dma_start(
                                out.ap()[ts(tt, 128),
                                         ds(dp * 1024 + d2 * 512, 512)],
                                yb[:])
    nc.compile()
    return nc


_NC_CACHE = None


def kernel(x, wq, wk, wv, wo, gate, adapter, freqs_cos, freqs_sin, mask,
           start_pos=0, **_unused):
    global _NC_CACHE
    if _NC_CACHE is None:
        _NC_CACHE = build()
    nc = _NC_CACHE
    xf = np.ascontiguousarray(np.asarray(x, np.float32).reshape(TOK, D))
    g = np.asarray(gate, np.float32).reshape(H)
    wof = np.asarray(wo, np.float32)
    in_maps = []
    for r in range(N_CORES):
        sl = slice(r * CH, (r + 1) * CH)
        in_maps.append({
            "x": xf,
            "wq": np.ascontiguousarray(np.asarray(wq, np.float32)[sl]),
            "wk": np.ascontiguousarray(np.asarray(wk, np.float32)[sl]),
            "wv": np.ascontiguousarray(np.asarray(wv, np.float32)[sl]),
            "wo_col": np.ascontiguousarray(wof[:, sl]),
            "gate": np.ascontiguousarray(
                g[r * H_LOC:(r + 1) * H_LOC].reshape(1, H_LOC)),
            "adapter": np.ascontiguousarray(
                np.asarray(adapter, np.float32).reshape(AL, D)),
            "fcos": np.ascontiguousarray(np.asarray(freqs_cos, np.float32)),
            "fsin": np.ascontiguousarray(np.asarray(freqs_sin, np.float32)),
        })
    res = bass_utils.run_bass_kernel_spmd(nc, in_maps,
                                          core_ids=list(range(N_CORES)))
    y = np.concatenate([res.results[r]["out"] for r in range(N_CORES)], axis=0)
    return y.reshape(B, S, D)


if __name__ == "__main__":
    nc = build()
    print("compiled ok, instrs:",
          sum(len(bb.instructions) for f in nc.m.functions for bb in f.blocks))
